# revision 1
# baseline (speedup 1.0000x reference)
"""Trainium2 Bass kernel for nn_CheapChannelV1 (dense_cnn).

Strategy (per core, pure data-parallel over batch):
  - The three channel-shuffle + 1x1-conv stages are linear, so they fold on the
    host into ONE 128x128 matrix M and bias b_tot:  res3 = M @ s + b_tot, where
    s = [s0;s1;s2;s3] are the four depthwise-conv branch outputs.
  - Level-0 depthwise conv (full res) folds INTO the matmul: 9 tap matmuls
    (K=32) reading shifted views of a zero-padded x0 strip tile.
  - Levels 1-3: max-pool on DVE, 3x3 depthwise conv on DVE in a 4-row-block
    strip layout; nearest-upsample folds into broadcast (step-0) rhs APs of the
    group matmuls.
  - 12 accumulating K=32 matmuls per 512-px chunk, spread across the four PE
    row groups via tile_position for concurrency.
  - Epilogue: exact Gelu on ACT (bias folded in), multiply-by-x on GPSIMD.
"""

import numpy as np

H = W = 256
CH = 128
NB = 8        # bands
SB = 8        # image rows per row-block per band (band covers 4*SB rows)
_DT = None    # lazy: mybir.dt.float32


def _shuf_cols(A, groups=8):
    # Returns A' with A' @ s == A @ channel_shuffle(s)
    Cin = A.shape[1]
    idx = np.arange(Cin)
    perm = (idx % groups) * (Cin // groups) + idx // groups
    Ap = np.zeros_like(A)
    Ap[:, perm] = A
    return Ap


def fold_weights(w_dw, b_dw, w_f1, b_f1, w_f2, b_f2, w_f3, b_f3):
    f8 = np.float64
    A1 = _shuf_cols(w_f1.astype(f8))
    A2 = _shuf_cols(w_f2.astype(f8))
    A3 = _shuf_cols(w_f3.astype(f8))
    A2a, A2b = A2[:, :64], A2[:, 64:]
    A3a, A3b = A3[:, :96], A3[:, 96:]
    M = np.zeros((128, 128), f8)
    M[:, 0:64] = A3a @ A2a @ A1
    M[:, 64:96] = A3a @ A2b
    M[:, 96:128] = A3b
    b_tot = A3a @ (A2a @ b_f1.astype(f8) + b_f2.astype(f8)) + b_f3.astype(f8)
    for g in range(4):
        b_tot = b_tot + M[:, 32 * g:32 * g + 32] @ b_dw[g].astype(f8)

    # W_all[p, t, o]: lhsT matrices, identical content per 32-partition group.
    W_all = np.zeros((128, 12, 128), np.float32)
    M0T = M[:, 0:32].T          # [32(c), 128(o)]
    w0 = w_dw[0].reshape(32, 9).astype(f8)
    for gp in range(4):
        rows = slice(32 * gp, 32 * gp + 32)
        for j in range(9):
            W_all[rows, j, :] = (M0T * w0[:, j:j + 1]).astype(np.float32)
        W_all[rows, 9, :] = M[:, 32:64].T.astype(np.float32)
        W_all[rows, 10, :] = M[:, 64:96].T.astype(np.float32)
        W_all[rows, 11, :] = M[:, 96:128].T.astype(np.float32)

    wdwp = np.zeros((128, 3, 9), np.float32)
    for g in (1, 2, 3):
        wdwp[:, g - 1, :] = np.tile(w_dw[g].reshape(32, 9), (4, 1)).astype(np.float32)

    return W_all, b_tot.astype(np.float32).reshape(128, 1), wdwp


def numpy_model(x, W_all, b_tot, wdwp):
    """Reference model of what the bass kernel computes (for one sample)."""
    from scipy.special import erf
    C, Hh, Ww = x.shape
    # level 0 conv via folded taps
    x0p = np.pad(x[:32], ((0, 0), (1, 1), (1, 1)))
    res3 = np.zeros((128, Hh, Ww), np.float32)
    for j in range(9):
        dy, dx = j // 3, j % 3
        lhsT = W_all[0:32, j, :]  # [32, 128]
        sh = x0p[:, dy:dy + Hh, dx:dx + Ww].reshape(32, -1)
        res3 += (lhsT.T @ sh).reshape(128, Hh, Ww)
    # pooled levels
    for g, k in ((1, 2), (2, 4), (3, 8)):
        xg = x[32 * g:32 * g + 32]
        p = xg.reshape(32, Hh // k, k, Ww // k, k).max(axis=(2, 4))
        pp = np.pad(p, ((0, 0), (1, 1), (1, 1)))
        conv = np.zeros_like(p)
        for j in range(9):
            dy, dx = j // 3, j % 3
            conv += wdwp[0:32, g - 1, j][:, None, None] * \
                pp[:, dy:dy + Hh // k, dx:dx + Ww // k]
        up = np.repeat(np.repeat(conv, k, axis=1), k, axis=2)
        lhsT = W_all[0:32, 8 + g, :]
        res3 += np.einsum('co,chw->ohw', lhsT, up)
    res3 = res3 + b_tot.reshape(128, 1, 1)
    g = 0.5 * res3 * (1.0 + erf(res3 / np.sqrt(2.0)))
    return (g * x).astype(np.float32)


_PROGRAM_CACHE = {}


def build_program(act_func_name="Gelu"):
    key = act_func_name
    if key in _PROGRAM_CACHE:
        return _PROGRAM_CACHE[key]

    import concourse.bacc as bacc
    import concourse.tile as tile
    import concourse.mybir as mybir

    f32 = mybir.dt.float32
    AOT = mybir.AluOpType
    act_func = getattr(mybir.ActivationFunctionType, act_func_name)

    nc = bacc.Bacc("TRN2", target_bir_lowering=False, debug=False)
    x_d = nc.dram_tensor("x", [CH, H, W], f32, kind="ExternalInput")
    wall_d = nc.dram_tensor("wall", [128, 12, 128], f32, kind="ExternalInput")
    btot_d = nc.dram_tensor("btot", [128, 1], f32, kind="ExternalInput")
    wdwp_d = nc.dram_tensor("wdwp", [128, 3, 9], f32, kind="ExternalInput")
    out_d = nc.dram_tensor("out", [CH, H, W], f32, kind="ExternalOutput")

    with tile.TileContext(nc) as tc:
        with tc.tile_pool(name="persist", bufs=1) as pers, \
             tc.tile_pool(name="xband", bufs=2) as xpool, \
             tc.tile_pool(name="x0strip", bufs=2) as x0pool, \
             tc.tile_pool(name="ptmp", bufs=1) as ptmp, \
             tc.tile_pool(name="convb", bufs=2) as cpool, \
             tc.tile_pool(name="psum", bufs=8, space="PSUM") as pspool, \
             tc.tile_pool(name="gout", bufs=4) as gpool, \
             tc.tile_pool(name="mout", bufs=4) as mpool:

            wall = pers.tile([128, 12, 128], f32)
            nc.sync.dma_start(wall[:], wall_d[:])
            btot = pers.tile([128, 1], f32)
            nc.sync.dma_start(btot[:], btot_d[:])
            wdwp = pers.tile([128, 3, 9], f32)
            nc.sync.dma_start(wdwp[:], wdwp_d[:])

            p1pad = pers.tile([128, 34, 130], f32)
            p2pad = pers.tile([128, 18, 66], f32)
            p3pad = pers.tile([128, 10, 34], f32)
            nc.vector.memset(p1pad[:], 0.0)
            nc.vector.memset(p2pad[:], 0.0)
            nc.vector.memset(p3pad[:], 0.0)

            # Top halos for pooled strips: strip rho's first conv row needs the
            # last pooled row of block rho-1, which only streams in at band 7.
            # Pool it up-front from a redundant load of the 8 image rows
            # preceding each block (r=1,2,3).
            xh = xpool.tile([128, 3, 8, 256], f32, tag="xband")
            for r in (1, 2, 3):
                nc.sync.dma_start(xh[:, r - 1], x_d[:, 64 * r - 8:64 * r, :])
            hhp1 = ptmp.tile([128, 3, 8, 128], f32, tag="hp1")
            nc.vector.tensor_tensor(
                hhp1[:], xh[:, :, :, 0::2], xh[:, :, :, 1::2], AOT.max)
            hvp1 = ptmp.tile([128, 3, 4, 128], f32, tag="vp1")
            nc.vector.tensor_tensor(
                hvp1[:], hhp1[:, :, 0::2, :], hhp1[:, :, 1::2, :], AOT.max)
            hhp2 = ptmp.tile([128, 3, 4, 64], f32, tag="hp2")
            nc.vector.tensor_tensor(
                hhp2[:], hvp1[:, :, :, 0::2], hvp1[:, :, :, 1::2], AOT.max)
            hvp2 = ptmp.tile([128, 3, 2, 64], f32, tag="vp2")
            nc.vector.tensor_tensor(
                hvp2[:], hhp2[:, :, 0::2, :], hhp2[:, :, 1::2, :], AOT.max)
            hhp3 = ptmp.tile([128, 3, 2, 32], f32, tag="hp3")
            nc.vector.tensor_tensor(
                hhp3[:], hvp2[:, :, :, 0::2], hvp2[:, :, :, 1::2], AOT.max)
            hvp3 = ptmp.tile([128, 3, 1, 32], f32, tag="vp3")
            nc.vector.tensor_tensor(
                hvp3[:], hhp3[:, :, 0::2, :], hhp3[:, :, 1::2, :], AOT.max)
            for r in (1, 2, 3):
                g = r * 32
                nc.sync.dma_start(p1pad[g:g + 32, 0:1, 1:129],
                                  hvp1[32:64, r - 1, 3:4, :])
                nc.sync.dma_start(p2pad[g:g + 32, 0:1, 1:65],
                                  hvp2[64:96, r - 1, 1:2, :])
                nc.sync.dma_start(p3pad[g:g + 32, 0:1, 1:33],
                                  hvp3[96:128, r - 1, 0:1, :])

            xband_prev = x0_prev = None

            for it in range(NB + 1):
                # ---------------- load + pool phase (band b = it) ------------
                if it < NB:
                    b = it
                    xband = xpool.tile([128, 4, SB, 256], f32)
                    for r in range(4):
                        nc.sync.dma_start(
                            xband[:, r],
                            x_d[:, 64 * r + SB * b: 64 * r + SB * b + SB, :])

                    x0 = x0pool.tile([128, SB + 2, 258], f32)
                    # zero the left/right pad columns (0 and 257)
                    nc.vector.memset(x0[:, :, ::257], 0.0)
                    for r in range(4):
                        lo = 64 * r + SB * b - 1
                        hi = lo + SB + 2
                        dlo, dhi = 0, SB + 2
                        if lo < 0:
                            nc.vector.memset(x0[32 * r:32 * r + 32, 0, :], 0.0)
                            dlo, lo = 1, 0
                        if hi > 256:
                            nc.vector.memset(
                                x0[32 * r:32 * r + 32, SB + 1, :], 0.0)
                            dhi, hi = SB + 1, 256
                        nc.sync.dma_start(
                            x0[32 * r:32 * r + 32, dlo:dhi, 1:257],
                            x_d[0:32, lo:hi, :])

                    # hierarchical 2x2 max pooling (channels 32..127)
                    # full-128-partition ops (lanes for unused channel groups
                    # compute junk for free; only the real slices get read)
                    hp1 = ptmp.tile([128, 4, SB, 128], f32)
                    nc.vector.tensor_tensor(
                        hp1[:], xband[:, :, :, 0::2],
                        xband[:, :, :, 1::2], AOT.max)
                    vp1 = ptmp.tile([128, 4, SB // 2, 128], f32)
                    nc.vector.tensor_tensor(
                        vp1[:], hp1[:, :, 0::2, :],
                        hp1[:, :, 1::2, :], AOT.max)
                    hp2 = ptmp.tile([128, 4, SB // 2, 64], f32)
                    nc.vector.tensor_tensor(
                        hp2[:], vp1[:, :, :, 0::2],
                        vp1[:, :, :, 1::2], AOT.max)
                    vp2 = ptmp.tile([128, 4, SB // 4, 64], f32)
                    nc.vector.tensor_tensor(
                        vp2[:], hp2[:, :, 0::2, :],
                        hp2[:, :, 1::2, :], AOT.max)
                    hp3 = ptmp.tile([128, 4, SB // 4, 32], f32)
                    nc.vector.tensor_tensor(
                        hp3[:], vp2[:, :, :, 0::2],
                        vp2[:, :, :, 1::2], AOT.max)
                    vp3 = ptmp.tile([128, 4, SB // 8, 32], f32)
                    nc.vector.tensor_tensor(
                        vp3[:], hp3[:, :, 0::2, :],
                        hp3[:, :, 1::2, :], AOT.max)

                    # scatter into persistent padded strip buffers
                    for r in range(4):
                        # pooled strips: strip rho=r lives at group r
                        g0 = r * 32
                        nc.sync.dma_start(
                            p1pad[g0:g0 + 32, 4 * b + 1:4 * b + 5, 1:129],
                            vp1[32:64, r])
                        nc.sync.dma_start(
                            p2pad[g0:g0 + 32, 2 * b + 1:2 * b + 3, 1:65],
                            vp2[64:96, r])
                        nc.sync.dma_start(
                            p3pad[g0:g0 + 32, b + 1:b + 2, 1:33],
                            vp3[96:128, r])
                        if b == 0 and r > 0:   # bottom halos of strip r-1
                            gm = (r - 1) * 32
                            nc.sync.dma_start(
                                p1pad[gm:gm + 32, 33:34, 1:129],
                                vp1[32:64, r, 0:1, :])
                            nc.sync.dma_start(
                                p2pad[gm:gm + 32, 17:18, 1:65],
                                vp2[64:96, r, 0:1, :])
                            nc.sync.dma_start(
                                p3pad[gm:gm + 32, 9:10, 1:33],
                                vp3[96:128, r, 0:1, :])

                # ---------------- compute phase (band bb = it-1) -------------
                if it > 0:
                    bb = it - 1
                    # pooled convs for this band's window (all 4 strips at once)
                    conv1 = cpool.tile([128, 4, 128], f32)
                    conv2 = cpool.tile([128, 2, 64], f32)
                    conv3 = cpool.tile([128, 1, 32], f32)
                    for j in range(9):
                        dy, dx = j // 3, j % 3
                        a1 = p1pad[:, 4 * bb + dy:4 * bb + dy + 4, dx:dx + 128]
                        a2 = p2pad[:, 2 * bb + dy:2 * bb + dy + 2, dx:dx + 64]
                        a3 = p3pad[:, bb + dy:bb + dy + 1, dx:dx + 32]
                        if j == 0:
                            nc.vector.tensor_scalar_mul(
                                conv1[:], a1, wdwp[:, 0, 0:1])
                            nc.vector.tensor_scalar_mul(
                                conv2[:], a2, wdwp[:, 1, 0:1])
                            nc.vector.tensor_scalar_mul(
                                conv3[:], a3, wdwp[:, 2, 0:1])
                        else:
                            nc.vector.scalar_tensor_tensor(
                                conv1[:], a1, wdwp[:, 0, j:j + 1], conv1[:],
                                AOT.mult, AOT.add)
                            nc.vector.scalar_tensor_tensor(
                                conv2[:], a2, wdwp[:, 1, j:j + 1], conv2[:],
                                AOT.mult, AOT.add)
                            nc.vector.scalar_tensor_tensor(
                                conv3[:], a3, wdwp[:, 2, j:j + 1], conv3[:],
                                AOT.mult, AOT.add)

                    for i in range(SB // 2):
                        pss = [pspool.tile([128, 2, 256], f32, tag="pschunk",
                                           name=f"ps_{bb}_{i}_{r}")
                               for r in range(4)]
                        for t in range(12):
                            for r in range(4):
                                g0 = 32 * r
                                if t < 3:
                                    lhsT = wall[g0:g0 + 32, 9 + t, :]
                                    if t == 0:
                                        rhs = conv1[g0:g0 + 32, i, :] \
                                            .unsqueeze(1).unsqueeze(3) \
                                            .broadcast_to([32, 2, 128, 2])
                                    elif t == 1:
                                        rhs = conv2[g0:g0 + 32, i // 2, :] \
                                            .unsqueeze(1).unsqueeze(3) \
                                            .broadcast_to([32, 2, 64, 4])
                                    else:
                                        rhs = conv3[g0:g0 + 32, 0, :] \
                                            .unsqueeze(1).unsqueeze(3) \
                                            .broadcast_to([32, 2, 32, 8])
                                else:
                                    j = t - 3
                                    dy, dx = j // 3, j % 3
                                    lhsT = wall[g0:g0 + 32, j, :]
                                    rhs = x0_prev[g0:g0 + 32,
                                                  2 * i + dy:2 * i + dy + 2,
                                                  dx:dx + 256]
                                nc.tensor.matmul(
                                    pss[r][:], lhsT, rhs,
                                    start=(t == 0), stop=(t == 11),
                                    tile_position=(g0, 0))
                        for r in range(4):
                            gt = gpool.tile([128, 2, 256], f32, tag="gchunk")
                            nc.scalar.activation(
                                gt[:], pss[r][:], act_func, bias=btot[:, 0:1])
                            mt = mpool.tile([128, 2, 256], f32, tag="mchunk")
                            nc.gpsimd.tensor_mul(
                                mt[:], gt[:],
                                xband_prev[:, r, 2 * i:2 * i + 2, :])
                            h = 64 * r + SB * bb + 2 * i
                            nc.sync.dma_start(out_d[:, h:h + 2, :], mt[:])

                if it < NB:
                    xband_prev, x0_prev = xband, x0

    nc.compile()
    _PROGRAM_CACHE[key] = nc
    return nc


def kernel(x, w_dw, b_dw, w_f1, b_f1, w_f2, b_f2, w_f3, b_f3):
    from concourse.bass_utils import run_bass_kernel_spmd

    x = np.asarray(x)
    B = x.shape[0]
    W_all, b_tot, wdwp = fold_weights(
        np.asarray(w_dw), np.asarray(b_dw), np.asarray(w_f1), np.asarray(b_f1),
        np.asarray(w_f2), np.asarray(b_f2), np.asarray(w_f3), np.asarray(b_f3))

    nc = build_program("Gelu")
    in_maps = [{"x": np.ascontiguousarray(x[i], dtype=np.float32),
                "wall": W_all, "btot": b_tot, "wdwp": wdwp}
               for i in range(B)]
    res = run_bass_kernel_spmd(nc, in_maps, list(range(B)))
    out = np.stack([res.results[i]["out"] for i in range(B)], axis=0)
    return out.astype(np.float32)



# revision 11
# speedup vs baseline: 1.5704x; 1.5704x over previous
"""Trainium2 Bass kernel for nn_CheapChannelV1 (dense_cnn).

Strategy (per core, pure data-parallel over batch):
  - The three channel-shuffle + 1x1-conv stages are linear, so they fold on the
    host into ONE 128x128 matrix M and bias b_tot:  res3 = M @ s + b_tot, where
    s = [s0;s1;s2;s3] are the four depthwise-conv branch outputs.
  - Level-0 depthwise conv (full res) folds INTO the matmul: 9 tap matmuls
    (K=32) reading shifted views of a zero-padded x0 strip tile.
  - Levels 1-3: max-pool on DVE, 3x3 depthwise conv on DVE in a 4-row-block
    strip layout; nearest-upsample folds into broadcast (step-0) rhs APs of the
    group matmuls.
  - 12 accumulating K=32 matmuls per 512-px chunk, spread across the four PE
    row groups via tile_position for concurrency.
  - Epilogue: exact Gelu on ACT (bias folded in), multiply-by-x on GPSIMD.
"""

import numpy as np

H = W = 256
CH = 128
NB = 8        # bands
SB = 8        # image rows per row-block per band (band covers 4*SB rows)
_DT = None    # lazy: mybir.dt.float32


def _shuf_cols(A, groups=8):
    # Returns A' with A' @ s == A @ channel_shuffle(s)
    Cin = A.shape[1]
    idx = np.arange(Cin)
    perm = (idx % groups) * (Cin // groups) + idx // groups
    Ap = np.zeros_like(A)
    Ap[:, perm] = A
    return Ap


def fold_weights(w_dw, b_dw, w_f1, b_f1, w_f2, b_f2, w_f3, b_f3):
    f8 = np.float64
    A1 = _shuf_cols(w_f1.astype(f8))
    A2 = _shuf_cols(w_f2.astype(f8))
    A3 = _shuf_cols(w_f3.astype(f8))
    A2a, A2b = A2[:, :64], A2[:, 64:]
    A3a, A3b = A3[:, :96], A3[:, 96:]
    M = np.zeros((128, 128), f8)
    M[:, 0:64] = A3a @ A2a @ A1
    M[:, 64:96] = A3a @ A2b
    M[:, 96:128] = A3b
    b_tot = A3a @ (A2a @ b_f1.astype(f8) + b_f2.astype(f8)) + b_f3.astype(f8)
    for g in range(4):
        b_tot = b_tot + M[:, 32 * g:32 * g + 32] @ b_dw[g].astype(f8)

    # W_all[p, t, o]: lhsT matrices, identical content per 32-partition group.
    W_all = np.zeros((128, 12, 128), np.float32)
    M0T = M[:, 0:32].T          # [32(c), 128(o)]
    w0 = w_dw[0].reshape(32, 9).astype(f8)
    for gp in range(4):
        rows = slice(32 * gp, 32 * gp + 32)
        for j in range(9):
            W_all[rows, j, :] = (M0T * w0[:, j:j + 1]).astype(np.float32)
        W_all[rows, 9, :] = M[:, 32:64].T.astype(np.float32)
        W_all[rows, 10, :] = M[:, 64:96].T.astype(np.float32)
        W_all[rows, 11, :] = M[:, 96:128].T.astype(np.float32)

    wdwp = np.zeros((128, 3, 9), np.float32)
    for g in (1, 2, 3):
        wdwp[:, g - 1, :] = np.tile(w_dw[g].reshape(32, 9), (4, 1)).astype(np.float32)

    return W_all, b_tot.astype(np.float32).reshape(128, 1), wdwp


def numpy_model(x, W_all, b_tot, wdwp):
    """Reference model of what the bass kernel computes (for one sample)."""
    from scipy.special import erf
    C, Hh, Ww = x.shape
    # level 0 conv via folded taps
    x0p = np.pad(x[:32], ((0, 0), (1, 1), (1, 1)))
    res3 = np.zeros((128, Hh, Ww), np.float32)
    for j in range(9):
        dy, dx = j // 3, j % 3
        lhsT = W_all[0:32, j, :]  # [32, 128]
        sh = x0p[:, dy:dy + Hh, dx:dx + Ww].reshape(32, -1)
        res3 += (lhsT.T @ sh).reshape(128, Hh, Ww)
    # pooled levels
    for g, k in ((1, 2), (2, 4), (3, 8)):
        xg = x[32 * g:32 * g + 32]
        p = xg.reshape(32, Hh // k, k, Ww // k, k).max(axis=(2, 4))
        pp = np.pad(p, ((0, 0), (1, 1), (1, 1)))
        conv = np.zeros_like(p)
        for j in range(9):
            dy, dx = j // 3, j % 3
            conv += wdwp[0:32, g - 1, j][:, None, None] * \
                pp[:, dy:dy + Hh // k, dx:dx + Ww // k]
        up = np.repeat(np.repeat(conv, k, axis=1), k, axis=2)
        lhsT = W_all[0:32, 8 + g, :]
        res3 += np.einsum('co,chw->ohw', lhsT, up)
    res3 = res3 + b_tot.reshape(128, 1, 1)
    g = 0.5 * res3 * (1.0 + erf(res3 / np.sqrt(2.0)))
    return (g * x).astype(np.float32)


_PROGRAM_CACHE = {}


def build_program(act_func_name="Gelu"):
    key = act_func_name
    if key in _PROGRAM_CACHE:
        return _PROGRAM_CACHE[key]

    import concourse.bacc as bacc
    import concourse.tile as tile
    import concourse.mybir as mybir

    f32 = mybir.dt.float32
    AOT = mybir.AluOpType
    act_func = getattr(mybir.ActivationFunctionType, act_func_name)

    f32r = mybir.dt.float32r

    nc = bacc.Bacc("TRN2", target_bir_lowering=False, debug=False)
    x_d = nc.dram_tensor("x", [CH, H, W], f32, kind="ExternalInput")
    wall_d = nc.dram_tensor("wall", [128, 12, 128], f32, kind="ExternalInput")
    btot_d = nc.dram_tensor("btot", [128, 1], f32, kind="ExternalInput")
    wdwp_d = nc.dram_tensor("wdwp", [128, 3, 9], f32, kind="ExternalInput")
    out_d = nc.dram_tensor("out", [CH, H, W], f32, kind="ExternalOutput")

    with tile.TileContext(nc) as tc:
        with tc.tile_pool(name="persist", bufs=1) as pers, \
             tc.tile_pool(name="xband", bufs=3) as xpool, \
             tc.tile_pool(name="x0strip", bufs=3) as x0pool, \
             tc.tile_pool(name="ptmp", bufs=1) as ptmp, \
             tc.tile_pool(name="convb", bufs=2) as cpool, \
             tc.tile_pool(name="psum", bufs=8, space="PSUM") as pspool, \
             tc.tile_pool(name="gout", bufs=3) as gpool, \
             tc.tile_pool(name="mout", bufs=2) as mpool:

            wall = pers.tile([128, 12, 128], f32)
            nc.sync.dma_start(wall[:], wall_d[:])
            btot = pers.tile([128, 1], f32)
            nc.sync.dma_start(btot[:], btot_d[:])
            wdwp = pers.tile([128, 3, 9], f32)
            nc.sync.dma_start(wdwp[:], wdwp_d[:])

            p1pad = pers.tile([128, 34, 130], f32)
            p2pad = pers.tile([128, 18, 66], f32)
            p3pad = pers.tile([128, 10, 34], f32)
            nc.vector.memset(p1pad[:], 0.0)
            nc.vector.memset(p2pad[:], 0.0)
            nc.vector.memset(p3pad[:], 0.0)

            # Top halos for pooled strips: strip rho's first conv row needs the
            # last pooled row of block rho-1, which only streams in at band 7.
            # Pool it up-front from a redundant load of the 8 image rows
            # preceding each block (r=1,2,3).
            xh = xpool.tile([128, 3, 8, 256], f32, tag="xband")
            for r in (1, 2, 3):
                nc.sync.dma_start(xh[:, r - 1], x_d[:, 64 * r - 8:64 * r, :])
            hhp1 = ptmp.tile([128, 3, 8, 128], f32, tag="hp1")
            nc.vector.tensor_tensor(
                hhp1[:], xh[:, :, :, 0::2], xh[:, :, :, 1::2], AOT.max)
            hvp1 = ptmp.tile([128, 3, 4, 128], f32, tag="vp1")
            nc.vector.tensor_tensor(
                hvp1[:], hhp1[:, :, 0::2, :], hhp1[:, :, 1::2, :], AOT.max)
            hhp2 = ptmp.tile([128, 3, 4, 64], f32, tag="hp2")
            nc.vector.tensor_tensor(
                hhp2[:], hvp1[:, :, :, 0::2], hvp1[:, :, :, 1::2], AOT.max)
            hvp2 = ptmp.tile([128, 3, 2, 64], f32, tag="vp2")
            nc.vector.tensor_tensor(
                hvp2[:], hhp2[:, :, 0::2, :], hhp2[:, :, 1::2, :], AOT.max)
            hhp3 = ptmp.tile([128, 3, 2, 32], f32, tag="hp3")
            nc.vector.tensor_tensor(
                hhp3[:], hvp2[:, :, :, 0::2], hvp2[:, :, :, 1::2], AOT.max)
            hvp3 = ptmp.tile([128, 3, 1, 32], f32, tag="vp3")
            nc.vector.tensor_tensor(
                hvp3[:], hhp3[:, :, 0::2, :], hhp3[:, :, 1::2, :], AOT.max)
            for r in (1, 2, 3):
                g = r * 32
                nc.sync.dma_start(p1pad[g:g + 32, 0:1, 1:129],
                                  hvp1[32:64, r - 1, 3:4, :])
                nc.sync.dma_start(p2pad[g:g + 32, 0:1, 1:65],
                                  hvp2[64:96, r - 1, 1:2, :])
                nc.sync.dma_start(p3pad[g:g + 32, 0:1, 1:33],
                                  hvp3[96:128, r - 1, 0:1, :])

            xbands = [None] * NB
            x0s = [None] * NB

            def load_band(b):
                xband = xpool.tile([128, 4, SB, 256], f32, name=f"xb_{b}",
                                   tag="xband")
                for r in range(4):
                    nc.sync.dma_start(
                        xband[:, r],
                        x_d[:, 64 * r + SB * b: 64 * r + SB * b + SB, :])

                x0 = x0pool.tile([128, SB + 2, 258], f32, name=f"x0_{b}",
                                 tag="x0")
                # zero the left/right pad columns (0 and 257)
                nc.vector.memset(x0[:, :, ::257], 0.0)
                for r in range(4):
                    lo = 64 * r + SB * b - 1
                    hi = lo + SB + 2
                    dlo, dhi = 0, SB + 2
                    if lo < 0:
                        nc.vector.memset(x0[32 * r:32 * r + 32, 0, :], 0.0)
                        dlo, lo = 1, 0
                    if hi > 256:
                        nc.vector.memset(
                            x0[32 * r:32 * r + 32, SB + 1, :], 0.0)
                        dhi, hi = SB + 1, 256
                    nc.sync.dma_start(
                        x0[32 * r:32 * r + 32, dlo:dhi, 1:257],
                        x_d[0:32, lo:hi, :])
                xbands[b], x0s[b] = xband, x0

            load_band(0)

            for it in range(NB + 1):
                # ---------------- prefetch next band ------------------------
                if it + 1 < NB:
                    load_band(it + 1)

                # ---------------- pool phase (band b = it) ------------------
                if it < NB:
                    b = it
                    xband = xbands[b]

                    # hierarchical 2x2 max pooling (channels 32..127)
                    # full-128-partition ops (lanes for unused channel groups
                    # compute junk for free; only the real slices get read)
                    hp1 = ptmp.tile([128, 4, SB, 128], f32)
                    nc.vector.tensor_tensor(
                        hp1[:], xband[:, :, :, 0::2],
                        xband[:, :, :, 1::2], AOT.max)
                    vp1 = ptmp.tile([128, 4, SB // 2, 128], f32)
                    nc.vector.tensor_tensor(
                        vp1[:], hp1[:, :, 0::2, :],
                        hp1[:, :, 1::2, :], AOT.max)
                    hp2 = ptmp.tile([128, 4, SB // 2, 64], f32)
                    nc.vector.tensor_tensor(
                        hp2[:], vp1[:, :, :, 0::2],
                        vp1[:, :, :, 1::2], AOT.max)
                    vp2 = ptmp.tile([128, 4, SB // 4, 64], f32)
                    nc.vector.tensor_tensor(
                        vp2[:], hp2[:, :, 0::2, :],
                        hp2[:, :, 1::2, :], AOT.max)
                    hp3 = ptmp.tile([128, 4, SB // 4, 32], f32)
                    nc.vector.tensor_tensor(
                        hp3[:], vp2[:, :, :, 0::2],
                        vp2[:, :, :, 1::2], AOT.max)
                    vp3 = ptmp.tile([128, 4, SB // 8, 32], f32)
                    nc.vector.tensor_tensor(
                        vp3[:], hp3[:, :, 0::2, :],
                        hp3[:, :, 1::2, :], AOT.max)

                    # scatter into persistent padded strip buffers
                    for r in range(4):
                        # pooled strips: strip rho=r lives at group r
                        g0 = r * 32
                        nc.sync.dma_start(
                            p1pad[g0:g0 + 32, 4 * b + 1:4 * b + 5, 1:129],
                            vp1[32:64, r])
                        nc.sync.dma_start(
                            p2pad[g0:g0 + 32, 2 * b + 1:2 * b + 3, 1:65],
                            vp2[64:96, r])
                        nc.sync.dma_start(
                            p3pad[g0:g0 + 32, b + 1:b + 2, 1:33],
                            vp3[96:128, r])
                        if b == 0 and r > 0:   # bottom halos of strip r-1
                            gm = (r - 1) * 32
                            nc.sync.dma_start(
                                p1pad[gm:gm + 32, 33:34, 1:129],
                                vp1[32:64, r, 0:1, :])
                            nc.sync.dma_start(
                                p2pad[gm:gm + 32, 17:18, 1:65],
                                vp2[64:96, r, 0:1, :])
                            nc.sync.dma_start(
                                p3pad[gm:gm + 32, 9:10, 1:33],
                                vp3[96:128, r, 0:1, :])

                # ---------------- compute phase (band bb = it-1) -------------
                if it > 0:
                    bb = it - 1
                    x0_prev, xband_prev = x0s[bb], xbands[bb]
                    # pooled convs for this band's window (all 4 strips at once)
                    conv1 = cpool.tile([128, 4, 128], f32)
                    conv2 = cpool.tile([128, 2, 64], f32)
                    conv3 = cpool.tile([128, 1, 32], f32)
                    for j in range(9):
                        dy, dx = j // 3, j % 3
                        a1 = p1pad[:, 4 * bb + dy:4 * bb + dy + 4, dx:dx + 128]
                        a2 = p2pad[:, 2 * bb + dy:2 * bb + dy + 2, dx:dx + 64]
                        a3 = p3pad[:, bb + dy:bb + dy + 1, dx:dx + 32]
                        if j == 0:
                            nc.vector.tensor_scalar_mul(
                                conv1[:], a1, wdwp[:, 0, 0:1])
                            nc.vector.tensor_scalar_mul(
                                conv2[:], a2, wdwp[:, 1, 0:1])
                            nc.vector.tensor_scalar_mul(
                                conv3[:], a3, wdwp[:, 2, 0:1])
                        else:
                            nc.vector.scalar_tensor_tensor(
                                conv1[:], a1, wdwp[:, 0, j:j + 1], conv1[:],
                                AOT.mult, AOT.add)
                            nc.vector.scalar_tensor_tensor(
                                conv2[:], a2, wdwp[:, 1, j:j + 1], conv2[:],
                                AOT.mult, AOT.add)
                            nc.vector.scalar_tensor_tensor(
                                conv3[:], a3, wdwp[:, 2, j:j + 1], conv3[:],
                                AOT.mult, AOT.add)

                    for i in range(SB // 2):
                        pss = [pspool.tile([128, 2, 256], f32, tag="pschunk",
                                           name=f"ps_{bb}_{i}_{r}")
                               for r in range(4)]
                        for t in range(12):
                            for r in range(4):
                                g0 = 32 * r
                                if t < 3:
                                    lhsT = wall[g0:g0 + 32, 9 + t, :]
                                    if t == 0:
                                        rhs = conv1[g0:g0 + 32, i, :] \
                                            .unsqueeze(1).unsqueeze(3) \
                                            .broadcast_to([32, 2, 128, 2])
                                    elif t == 1:
                                        rhs = conv2[g0:g0 + 32, i // 2, :] \
                                            .unsqueeze(1).unsqueeze(3) \
                                            .broadcast_to([32, 2, 64, 4])
                                    else:
                                        rhs = conv3[g0:g0 + 32, 0, :] \
                                            .unsqueeze(1).unsqueeze(3) \
                                            .broadcast_to([32, 2, 32, 8])
                                else:
                                    j = t - 3
                                    dy, dx = j // 3, j % 3
                                    lhsT = wall[g0:g0 + 32, j, :] \
                                        .bitcast(f32r)
                                    rhs = x0_prev[g0:g0 + 32,
                                                  2 * i + dy:2 * i + dy + 2,
                                                  dx:dx + 256].bitcast(f32r)
                                nc.tensor.matmul(
                                    pss[r][:], lhsT, rhs,
                                    start=(t == 0), stop=(t == 11),
                                    tile_position=(g0, 0))
                        for r in range(4):
                            gt = gpool.tile([128, 2, 256], f32, tag="gchunk")
                            nc.scalar.activation(
                                gt[:], pss[r][:], act_func, bias=btot[:, 0:1])
                            mt = mpool.tile([128, 2, 256], f32, tag="mchunk")
                            nc.gpsimd.tensor_mul(
                                mt[:], gt[:],
                                xband_prev[:, r, 2 * i:2 * i + 2, :])
                            h = 64 * r + SB * bb + 2 * i
                            nc.sync.dma_start(out_d[:, h:h + 2, :], mt[:])

    nc.compile()
    _PROGRAM_CACHE[key] = nc
    return nc


def kernel(x, w_dw, b_dw, w_f1, b_f1, w_f2, b_f2, w_f3, b_f3):
    from concourse.bass_utils import run_bass_kernel_spmd

    x = np.asarray(x)
    B = x.shape[0]
    W_all, b_tot, wdwp = fold_weights(
        np.asarray(w_dw), np.asarray(b_dw), np.asarray(w_f1), np.asarray(b_f1),
        np.asarray(w_f2), np.asarray(b_f2), np.asarray(w_f3), np.asarray(b_f3))

    nc = build_program("Gelu")
    in_maps = [{"x": np.ascontiguousarray(x[i], dtype=np.float32),
                "wall": W_all, "btot": b_tot, "wdwp": wdwp}
               for i in range(B)]
    res = run_bass_kernel_spmd(nc, in_maps, list(range(B)))
    out = np.stack([res.results[i]["out"] for i in range(B)], axis=0)
    return out.astype(np.float32)



# revision 13
# speedup vs baseline: 1.6298x; 1.0378x over previous
"""Trainium2 Bass kernel for nn_CheapChannelV1 (dense_cnn).

Per core (pure data-parallel over batch, one sample per core):
  - All linear stages fold on the host into one 128x128 matrix M + bias.
  - Image processed in 16 bands of 16 rows; each band -> 8 chunks of
    2 rows x 256 cols (one PSUM bank each).
  - Level-0 depthwise conv folds into 3 K=96 bf16 matmuls over three
    row-shifted copies of a host-padded bf16 level-0 tensor (partitions
    32*dy+c), one matmul per tap column dx.
  - Levels 1-3: hierarchical max pool (bf16) into persistent padded
    canvases (all on partitions 32:64 so every pooled matmul stays at
    one PE tile position - bf16 accumulation groups crash HW when they
    hop tile positions), 3x3 depthwise conv on DVE, then 3 K=32 bf16
    matmuls with broadcast (nearest-upsample) rhs access patterns.
  - Epilogue: exact Gelu+bias on ACT (PSUM->SBUF, bf16 out), multiply
    by x on the Pool engine, store f32.
  - Pipeline: loads one band ahead; convs split into main rows (own
    band's pooled rows only) and a tail row (next band's first pooled
    row) so the PE never stalls on the pool->scatter->conv chain.
"""

import numpy as np

H = W = 256
CH = 128
NB = 16       # bands
RB = 16       # rows per band
NCK = RB // 2  # chunks per band


def _shuf_cols(A, groups=8):
    # Returns A' with A' @ s == A @ channel_shuffle(s)
    Cin = A.shape[1]
    idx = np.arange(Cin)
    perm = (idx % groups) * (Cin // groups) + idx // groups
    Ap = np.zeros_like(A)
    Ap[:, perm] = A
    return Ap


def fold_weights(w_dw, b_dw, w_f1, b_f1, w_f2, b_f2, w_f3, b_f3):
    import ml_dtypes
    f8 = np.float64
    A1 = _shuf_cols(w_f1.astype(f8))
    A2 = _shuf_cols(w_f2.astype(f8))
    A3 = _shuf_cols(w_f3.astype(f8))
    A2a, A2b = A2[:, :64], A2[:, 64:]
    A3a, A3b = A3[:, :96], A3[:, 96:]
    M = np.zeros((128, 128), f8)
    M[:, 0:64] = A3a @ A2a @ A1
    M[:, 64:96] = A3a @ A2b
    M[:, 96:128] = A3b
    b_tot = A3a @ (A2a @ b_f1.astype(f8) + b_f2.astype(f8)) + b_f3.astype(f8)
    for g in range(4):
        b_tot = b_tot + M[:, 32 * g:32 * g + 32] @ b_dw[g].astype(f8)

    # w96[32*dy+c, dx, o] = M[o, c] * w0[c, 3*dy+dx]  (level-0 tap lhsT)
    M0T = M[:, 0:32].T          # [32(c), 128(o)]
    w0 = w_dw[0].reshape(32, 9).astype(f8)
    w96 = np.zeros((128, 3, 128), np.float32)
    for dy in range(3):
        for dx in range(3):
            w96[32 * dy:32 * dy + 32, dx, :] = \
                (M0T * w0[:, 3 * dy + dx:3 * dy + dx + 1]).astype(np.float32)
    w96 = w96.astype(ml_dtypes.bfloat16)

    # All pooled-branch data lives on partitions 32:64 (single PE tile
    # position): wpool[32+c, g-1, o] = M[o, 32*g+c]
    wpool = np.zeros((128, 3, 128), np.float32)
    for g in (1, 2, 3):
        wpool[32:64, g - 1, :] = M[:, 32 * g:32 * g + 32].T \
            .astype(np.float32)
    wpool = wpool.astype(ml_dtypes.bfloat16)

    # wdwp9[32+c, g-1, j] = w_dw[g][c].flat[j]  (DVE conv scalars)
    wdwp9 = np.zeros((128, 3, 9), np.float32)
    for g in (1, 2, 3):
        wdwp9[32:64, g - 1, :] = w_dw[g].reshape(32, 9)

    return (w96, wpool, wdwp9,
            b_tot.astype(np.float32).reshape(128, 1))


_PROGRAM_CACHE = {}


def build_program(act_func_name="Gelu", do_lvl0=True, do_pooled=True,
                  do_mult=True):
    key = (act_func_name, do_lvl0, do_pooled, do_mult)
    if key in _PROGRAM_CACHE:
        return _PROGRAM_CACHE[key]

    import concourse.bacc as bacc
    import concourse.tile as tile
    import concourse.mybir as mybir

    f32 = mybir.dt.float32
    bf16 = mybir.dt.bfloat16
    f32r = mybir.dt.float32r
    AOT = mybir.AluOpType
    act_func = getattr(mybir.ActivationFunctionType, act_func_name)

    nc = bacc.Bacc("TRN2", target_bir_lowering=False, debug=False)
    # host-prepared bf16 copy of x (the f32 original is never loaded)
    xbf_d = nc.dram_tensor("xbf", [CH, H, W], bf16, kind="ExternalInput")
    # host-prepared zero-padded bf16 copy of channels 0:32
    # x0pad[c, 1+r, 1+col] = bf16(x[c, r, col])
    x0pad_d = nc.dram_tensor("x0pad", [32, H + 2, W + 2], bf16,
                             kind="ExternalInput")
    w96_d = nc.dram_tensor("w96", [128, 3, 128], bf16, kind="ExternalInput")
    wpool_d = nc.dram_tensor("wpool", [128, 3, 128], bf16,
                             kind="ExternalInput")
    wdwp_d = nc.dram_tensor("wdwp", [128, 3, 9], f32, kind="ExternalInput")
    btot_d = nc.dram_tensor("btot", [128, 1], f32, kind="ExternalInput")
    out_d = nc.dram_tensor("out", [CH, H, W], f32, kind="ExternalOutput")

    with tile.TileContext(nc) as tc:
        with tc.tile_pool(name="persist", bufs=1) as pers, \
             tc.tile_pool(name="xband", bufs=3) as xpool, \
             tc.tile_pool(name="x0c", bufs=2) as x0pool, \
             tc.tile_pool(name="ptmp", bufs=2) as ptmp, \
             tc.tile_pool(name="convb", bufs=2) as cpool, \
             tc.tile_pool(name="psum", bufs=8, space="PSUM") as pspool, \
             tc.tile_pool(name="gout", bufs=4) as gpool, \
             tc.tile_pool(name="mout", bufs=4) as mpool:

            w96t = pers.tile([128, 3, 128], bf16)
            nc.sync.dma_start(w96t[:], w96_d[:])
            wpoolt = pers.tile([128, 3, 128], bf16)
            nc.sync.dma_start(wpoolt[:], wpool_d[:])
            wdwp = pers.tile([128, 3, 9], f32)
            nc.sync.dma_start(wdwp[:], wdwp_d[:])
            btot = pers.tile([128, 1], f32)
            nc.sync.dma_start(btot[:], btot_d[:])

            # Pooled-map canvases, all meaningful on partitions 32:64 so
            # every pooled matmul stays at PE tile position (32, 0): a bf16
            # accumulation group crashes HW if it hops tile positions.
            # Pooled row t -> canvas row t+1, col c+1 (1-px zero pad).
            pm1 = pers.tile([128, 130, 130], bf16)
            pm2 = pers.tile([128, 66, 66], bf16)
            pm3 = pers.tile([128, 34, 34], bf16)
            nc.vector.memset(pm1[:], 0.0)
            nc.vector.memset(pm2[:], 0.0)
            nc.vector.memset(pm3[:], 0.0)

            xbands = [None] * NB
            x0cs = [None] * NB

            def load_band(b):
                xband = xpool.tile([128, RB, 256], bf16, name=f"xb_{b}",
                                   tag="xband")
                nc.sync.dma_start(xband[:], xbf_d[:, RB * b: RB * b + RB, :])
                xbands[b] = xband

            def pool_band(b):
                xband = xbands[b]
                hp1 = ptmp.tile([128, RB, 128], bf16, tag="hp1")
                nc.vector.tensor_tensor(
                    hp1[:], xband[:, :, 0::2], xband[:, :, 1::2], AOT.max)
                vp1 = ptmp.tile([128, RB // 2, 128], bf16, tag="vp1")
                nc.vector.tensor_tensor(
                    vp1[:], hp1[:, 0::2, :], hp1[:, 1::2, :], AOT.max)
                hp2 = ptmp.tile([128, RB // 2, 64], bf16, tag="hp2")
                nc.vector.tensor_tensor(
                    hp2[:], vp1[:, :, 0::2], vp1[:, :, 1::2], AOT.max)
                vp2 = ptmp.tile([128, RB // 4, 64], bf16, tag="vp2")
                nc.vector.tensor_tensor(
                    vp2[:], hp2[:, 0::2, :], hp2[:, 1::2, :], AOT.max)
                hp3 = ptmp.tile([128, RB // 4, 32], bf16, tag="hp3")
                nc.vector.tensor_tensor(
                    hp3[:], vp2[:, :, 0::2], vp2[:, :, 1::2], AOT.max)
                vp3 = ptmp.tile([128, RB // 8, 32], bf16, tag="vp3")
                nc.vector.tensor_tensor(
                    vp3[:], hp3[:, 0::2, :], hp3[:, 1::2, :], AOT.max)
                # scatter the real slices into the canvases (the DMA also
                # moves lvl2/lvl3 values onto partitions 32:64)
                nc.sync.dma_start(
                    pm1[32:64, 8 * b + 1:8 * b + 9, 1:129], vp1[32:64])
                nc.sync.dma_start(
                    pm2[32:64, 4 * b + 1:4 * b + 5, 1:65], vp2[64:96])
                nc.sync.dma_start(
                    pm3[32:64, 2 * b + 1:2 * b + 3, 1:33], vp3[96:128])

            def build_x0c(b):
                # x0c[32*dy+c, Y, X] = x0pad[c, 16*b + Y + dy, X]
                # Three row-displaced copies loaded straight from the
                # host-padded bf16 tensor (pads included).
                x0c = x0pool.tile([128, RB, 258], bf16, name=f"x0c_{b}",
                                  tag="x0c")
                for dy in range(3):
                    lo = RB * b + dy
                    nc.sync.dma_start(
                        x0c[32 * dy:32 * dy + 32, :, :],
                        x0pad_d[:, lo:lo + RB, :])
                x0cs[b] = x0c

            convs = [None] * NB

            def conv_rows(bb, lo1, n1, lo2, n2, lo3, n3, tiles=None):
                # rows [loK, loK+nK) of the three depthwise conv outputs
                if tiles is None:
                    conv1 = cpool.tile([128, 8, 128], bf16, tag="c1")
                    conv2 = cpool.tile([128, 4, 64], bf16, tag="c2")
                    conv3 = cpool.tile([128, 2, 32], bf16, tag="c3")
                else:
                    conv1, conv2, conv3 = tiles
                for j in range(9 if do_pooled else 0):
                    dy, dx = j // 3, j % 3
                    work = (
                        (conv1[:, lo1:lo1 + n1, :],
                         pm1[:, 8 * bb + lo1 + dy:8 * bb + lo1 + dy + n1,
                             dx:dx + 128], 0),
                        (conv2[:, lo2:lo2 + n2, :],
                         pm2[:, 4 * bb + lo2 + dy:4 * bb + lo2 + dy + n2,
                             dx:dx + 64], 1),
                        (conv3[:, lo3:lo3 + n3, :],
                         pm3[:, 2 * bb + lo3 + dy:2 * bb + lo3 + dy + n3,
                             dx:dx + 32], 2),
                    )
                    for out, win, lv in work:
                        if j == 0:
                            nc.vector.tensor_scalar_mul(out, win,
                                                        wdwp[:, lv, 0:1])
                        else:
                            nc.vector.scalar_tensor_tensor(
                                out, win, wdwp[:, lv, j:j + 1], out,
                                AOT.mult, AOT.add)
                return conv1, conv2, conv3

            def convs_main(bb):
                # rows whose 3x3 window stays inside band bb's pooled rows
                convs[bb] = conv_rows(bb, 0, 7, 0, 3, 0, 1)

            def convs_tail(bb):
                # last row per level: needs band bb+1's first pooled row
                conv_rows(bb, 7, 1, 3, 1, 1, 1, tiles=convs[bb])

            def compute_band(bb):
                x0c, xband = x0cs[bb], xbands[bb]
                conv1, conv2, conv3 = convs[bb]

                for i in range(NCK):
                    ps = pspool.tile([128, 2, 256], f32, tag="ps",
                                     name=f"ps_{bb}_{i}")
                    if do_lvl0:
                        for dx in range(3):
                            nc.tensor.matmul(
                                ps[:],
                                w96t[0:96, dx, :],
                                x0c[0:96, 2 * i:2 * i + 2, dx:dx + 256],
                                start=(dx == 0), stop=False)
                    if do_pooled:
                        nc.tensor.matmul(
                            ps[:], wpoolt[32:64, 0, :],
                            conv1[32:64, i, :].unsqueeze(1).unsqueeze(3)
                            .broadcast_to([32, 2, 128, 2]),
                            start=not do_lvl0, stop=False,
                            tile_position=(32, 0))
                        nc.tensor.matmul(
                            ps[:], wpoolt[32:64, 1, :],
                            conv2[32:64, i // 2, :].unsqueeze(1).unsqueeze(3)
                            .broadcast_to([32, 2, 64, 4]),
                            start=False, stop=False, tile_position=(32, 0))
                        nc.tensor.matmul(
                            ps[:], wpoolt[32:64, 2, :],
                            conv3[32:64, i // 4, :].unsqueeze(1).unsqueeze(3)
                            .broadcast_to([32, 2, 32, 8]),
                            start=False, stop=True, tile_position=(32, 0))
                    else:
                        nc.tensor.matmul(
                            ps[:],
                            w96t[0:96, 0, :],
                            x0c[0:96, 2 * i:2 * i + 2, 0:256],
                            start=False, stop=True)

                    gt = gpool.tile([128, 2, 256], bf16, tag="g")
                    nc.scalar.activation(gt[:], ps[:], act_func,
                                         bias=btot[:, 0:1])
                    mt = mpool.tile([128, 2, 256], f32, tag="m")
                    if not do_mult:
                        nc.gpsimd.tensor_copy(mt[:], gt[:])
                    elif i in (0, 2, 4):
                        # DVE drains these fast and stays clear of the
                        # band boundary; Pool (slow sw multiply) takes
                        # the rest so neither engine rate-limits.
                        nc.vector.tensor_tensor(
                            mt[:], gt[:], xband[:, 2 * i:2 * i + 2, :],
                            AOT.mult)
                    else:
                        nc.gpsimd.tensor_mul(
                            mt[:], gt[:], xband[:, 2 * i:2 * i + 2, :])
                    h = RB * bb + 2 * i
                    nc.sync.dma_start(out_d[:, h:h + 2, :], mt[:])

            load_band(0)
            load_band(1)
            pool_band(0)
            convs_main(0)
            build_x0c(0)
            for it in range(1, NB + 1):
                if it + 1 < NB:
                    load_band(it + 1)
                if it < NB:
                    pool_band(it)
                    convs_tail(it - 1)  # needs pool(it)'s first rows only
                    convs_main(it)      # overlaps with chunks(it-1) below
                    build_x0c(it)
                else:
                    convs_tail(it - 1)
                compute_band(it - 1)

    nc.compile()
    _PROGRAM_CACHE[key] = nc
    return nc


def make_in_maps(x, w_dw, b_dw, w_f1, b_f1, w_f2, b_f2, w_f3, b_f3):
    import ml_dtypes
    x = np.asarray(x)
    B = x.shape[0]
    w96, wpool, wdwp9, btot = fold_weights(
        np.asarray(w_dw), np.asarray(b_dw), np.asarray(w_f1),
        np.asarray(b_f1), np.asarray(w_f2), np.asarray(b_f2),
        np.asarray(w_f3), np.asarray(b_f3))
    maps = []
    for i in range(B):
        x0pad = np.zeros((32, H + 2, W + 2), ml_dtypes.bfloat16)
        x0pad[:, 1:H + 1, 1:W + 1] = x[i, 0:32].astype(ml_dtypes.bfloat16)
        maps.append({"xbf": x[i].astype(ml_dtypes.bfloat16),
                     "x0pad": x0pad, "w96": w96, "wpool": wpool,
                     "wdwp": wdwp9, "btot": btot})
    return maps


def kernel(x, w_dw, b_dw, w_f1, b_f1, w_f2, b_f2, w_f3, b_f3):
    from concourse.bass_utils import run_bass_kernel_spmd

    x = np.asarray(x)
    B = x.shape[0]
    in_maps = make_in_maps(x, w_dw, b_dw, w_f1, b_f1, w_f2, b_f2,
                           w_f3, b_f3)
    nc = build_program("Gelu")
    res = run_bass_kernel_spmd(nc, in_maps, list(range(B)))
    out = np.stack([res.results[i]["out"] for i in range(B)], axis=0)
    return out.astype(np.float32)
